# revision 33
# baseline (speedup 1.0000x reference)
"""Llama GQA attention (B=1, Q=1024, PAST=3072, HID=4096, NH=32, NKV=8, HD=128)
tensor-parallel over heads across 8 NeuronCores.

Per core c: kv head c, query heads 4c..4c+3. Each core computes its partial
o_proj contribution [1024, 4096] in bf16; the host sums the 8 partials in f32.

Per-core layout strategy (v2 — DVE off the critical path):
  - QKV proj: out[seq, :] tiles via lhsT = hsT k-tile (stationary), rhs = W.T.
  - RoPE: ACT copies PSUM->SBUF bf16, DVE bf16 muls (rotate-half via strided
    3D APs), PE transpose to [d, seq], ACT copy to qt/kt buffers.
  - scores TRANSPOSED: scoresT[kv, seq] = K_T_tile.T @ qT. Softmax without
    max-subtraction (constant -20 shift cancels per-row). exp on ACT directly
    from PSUM. Causality by STRUCTURE: fully-masked kv tiles skipped, fully
    unmasked tiles need no mask at all, and only the 4 boundary tiles per
    (group, head) get a post-exp 0/1 multiplicative mask on DVE (bf16 2x).
  - denominator: PSUM-accumulated [1, 512] ones-matmul per kv tile (PE),
    reciprocal_approx_fast (DVE), broadcast via f32r ones-row matmul,
    normalization mul on DVE reading the broadcast from PSUM.
  - attn: attnT[d, seq] accumulated via lhsT = v_kt [kv, d], rhs = pT_kt.
  - o_proj: out[seq, hid] tiles, lhsT = attnT head-slice, 4-head accumulate,
    ACT copy PSUM->SBUF bf16, DMA out bf16.
"""

import math
import numpy as np
import ml_dtypes

import bass_rust
import concourse.bass as bass
import concourse.mybir as mybir
import concourse.tile as tile
from concourse.vector_clock import ScopedClock
from concourse.masks import make_identity
from concourse.bass_utils import run_bass_kernel_spmd

# ---------------------------------------------------------------------------
# Workaround: walrus in this image rejects >1 sem wait on CTRL-class
# instructions (Drain/NoOp). TileContext's tail drain waits on every touched
# logical processor. Split the waits across preceding sync-engine nops.
MAX_WAITS = 1


def _split_waits(nc, inst):
    si = inst.ins.sync_info
    if si is None:
        return
    waits = list(si.on_wait)
    if len(waits) <= MAX_WAITS:
        return
    inst.ins.sync_info = bass_rust.SyncInfo(
        on_wait=waits[:MAX_WAITS], on_update=list(si.on_update)
    )
    rest = waits[MAX_WAITS:]
    while rest:
        extra = nc.sync.nop(nofuse=True)
        extra.ins.sync_info = bass_rust.SyncInfo(on_wait=rest[:MAX_WAITS], on_update=[])
        rest = rest[MAX_WAITS:]


def _drain_and_barrier_split(self, tick_clock, wait_clock):
    nc = self.nc
    carrier = nc.sync.nop(nofuse=True)
    wait_clock.add_sem_waits(carrier.ins, ScopedClock({None: tick_clock.global_clock}))
    _split_waits(nc, carrier)
    nc.sync.drain()
    nc.all_engine_barrier()
    popped = nc._tile_sem_poison_stack.pop()
    assert popped is self._sem_poison
    nc.clear_and_free_semaphores(list(self.sems.allocated().values()))
    nc.all_engine_barrier()


tile.TileContext._drain_and_barrier = _drain_and_barrier_split
# ---------------------------------------------------------------------------

# ---------------------------------------------------------------------------
# General wait-cap legalization: this walrus rejects instructions carrying
# more than a couple of sem waits. Post-process the BIR JSON: hoist overflow
# waits onto engine-matched NoOps inserted immediately before the offender
# (same engine queue -> same ordering semantics).
import json as _json

_CTRL_OPS = {"NoOp", "Drain", "EventSemaphore"}
_CAP_CTRL = 1
_CAP_OTHER = 1
_orig_to_json_bytes = bass.Bass.to_json_bytes


def _legalized_to_json_bytes(self, *a, **k):
    raw = _orig_to_json_bytes(self, *a, **k)
    m = _json.loads(raw)
    ctr = [0]
    changed = False
    for fn in m.get("functions", []):
        for blk in fn.get("blocks", []):
            insts = blk.get("instructions", [])
            out = []
            for ins in insts:
                si = ins.get("sync_info")
                if si:
                    waits = si.get("on_wait") or []
                    cap = _CAP_CTRL if ins.get("opcode") in _CTRL_OPS else _CAP_OTHER
                    if len(waits) > cap:
                        changed = True
                        rest = waits[:-cap]
                        si["on_wait"] = waits[-cap:]
                        while rest:
                            ctr[0] += 1
                            out.append({
                                "debug": ins.get("debug", 0),
                                "engine": ins["engine"],
                                "ins": [], "outs": [],
                                "name": f"{ins['name']}_lw{ctr[0]}",
                                "opcode": "NoOp",
                                "sync_info": {"on_wait": rest[:_CAP_CTRL],
                                              "on_update": []},
                            })
                            rest = rest[_CAP_CTRL:]
                out.append(ins)
            blk["instructions"] = out
    if not changed:
        return raw
    return _json.dumps(m).encode()


bass.Bass.to_json_bytes = _legalized_to_json_bytes
# ---------------------------------------------------------------------------


B, Q, PAST, HID = 1, 1024, 3072, 4096
NH, NKV, HD = 32, 8, 128
KV = PAST + Q           # 4096
NCORES = 8
HPC = NH // NCORES      # 4 query heads per core
ROPE_THETA = 10000.0
EXP_SHIFT = -20.0       # constant softmax shift (cancels exactly per row)

F32 = mybir.dt.float32
F32R = mybir.dt.float32r
BF16 = mybir.dt.bfloat16

N_KT = KV // 128        # 32 kv tiles
N_ST = Q // 128         # 8 seq tiles
N_HK = HID // 128       # 32 hid k-tiles
GRP = 512               # seq group width for scores/attn
N_G = Q // GRP          # 2 groups
N_PV = PAST // 128      # 24 past-v tiles

# causal structure at GRP x 128 granularity:
#   group g fully attends kv tiles [0, 24+4g), boundary tiles [24+4g, 28+4g),
#   and (g=0 only) tiles >= 28 are fully masked -> skipped.
def _n_kt(g):
    return 28 + 4 * g


def _boundary(g, kt):
    return 24 + 4 * g <= kt


LAST_RESULTS = None     # test harness reads exec_time_ns from here


def _build_program():
    nc = bass.Bass()
    # hs/weight chunks are stored k-tile-contiguous in DRAM (one 128-row block
    # per k-tile) so every dma_start reads one fully contiguous region
    hst = nc.declare_dram_parameter("hst", [N_HK * 128, Q], BF16, isOutput=False)
    wqt = nc.declare_dram_parameter("wqt", [N_HK * 128, HPC * 128], BF16, isOutput=False)
    wkvt = nc.declare_dram_parameter("wkvt", [N_HK * 128, 256], BF16, isOutput=False)
    pastkt = nc.declare_dram_parameter("pastkt", [128, PAST], BF16, isOutput=False)
    pastv = nc.declare_dram_parameter("pastv", [128, PAST], BF16, isOutput=False)
    mask01 = nc.declare_dram_parameter("mask01", [128, 8 * GRP], BF16, isOutput=False)
    cosq4 = nc.declare_dram_parameter("cosq4", [128, N_ST * HPC * HD], BF16, isOutput=False)
    sinq4 = nc.declare_dram_parameter("sinq4", [128, N_ST * HPC * HD], BF16, isOutput=False)
    cosk = nc.declare_dram_parameter("cosk", [128, N_ST * HD], BF16, isOutput=False)
    sink = nc.declare_dram_parameter("sink", [128, N_ST * HD], BF16, isOutput=False)
    wot = nc.declare_dram_parameter("wot", [HPC * 128, HID], BF16, isOutput=False)
    outp = nc.declare_dram_parameter("outp", [Q, HID], BF16, isOutput=True)

    with tile.TileContext(nc) as tc:
        with (
            tc.tile_pool(name="const", bufs=1) as cpool,
            tc.tile_pool(name="kvres", bufs=1) as kvpool,
            tc.tile_pool(name="qt", bufs=1) as qtpool,
            tc.tile_pool(name="attn", bufs=1) as apool,
        ):
            ident = cpool.tile([128, 128], BF16)
            make_identity(nc, ident[:])
            ones_col = cpool.tile([128, 1], BF16)
            nc.vector.memset(ones_col[:], 1.0)
            shift_sb = cpool.tile([128, 1], F32)
            nc.vector.memset(shift_sb[:], EXP_SHIFT)

            # K_T [128 d, KV] bf16; V packed [128 kv-sub, N_KT*128 d]
            # (DMAs issued inside stage 1, after the critical hs/wq loads)
            kt_sb = kvpool.tile([128, KV], BF16)
            v_sb = kvpool.tile([128, N_KT * 128], BF16)

            # qT all heads [128 d, HPC*Q] bf16 (head-major); same for attnT
            qt_all = qtpool.tile([128, HPC * Q], BF16)
            at_all = apool.tile([128, HPC * Q], BF16)
            qt3 = qt_all[:].rearrange("p (h q) -> p h q", h=HPC)
            at3 = at_all[:].rearrange("p (h q) -> p h q", h=HPC)

            # ---------------- stage 1: QKV projection + RoPE ----------------
            with (
                tc.tile_pool(name="hsw", bufs=1) as hspool,
                tc.tile_pool(name="rope", bufs=3) as rpool,
                tc.tile_pool(name="tps", bufs=2, space="PSUM") as tps,
                tc.tile_pool(name="tpk", bufs=2, space="PSUM") as tpkps,
                tc.tile_pool(name="qkvps", bufs=2, space="PSUM") as qkvps,
            ):
                cos4_sb = hspool.tile([128, N_ST * HPC * HD], BF16)
                sin4_sb = hspool.tile([128, N_ST * HPC * HD], BF16)
                cosk_sb = hspool.tile([128, N_ST * HD], BF16)
                sink_sb = hspool.tile([128, N_ST * HD], BF16)
                hs_sb = hspool.tile([128, N_HK * Q], BF16)
                wq_sb = hspool.tile([128, N_HK * HPC * 128], BF16)
                wkv_sb = hspool.tile([128, N_HK * 256], BF16)
                # per-k-tile interleaved loads so the (st=0, k) matmul stream
                # can start as soon as the first chunks land
                for k in range(N_HK):
                    nc.sync.dma_start(hs_sb[:, k * Q:(k + 1) * Q],
                                      hst[k * 128:(k + 1) * 128, :])
                    nc.sync.dma_start(
                        wq_sb[:, k * HPC * 128:(k + 1) * HPC * 128],
                        wqt[k * 128:(k + 1) * 128, :],
                    )
                    nc.sync.dma_start(wkv_sb[:, k * 256:(k + 1) * 256],
                                      wkvt[k * 128:(k + 1) * 128, :])
                nc.sync.dma_start(cos4_sb[:], cosq4[:])
                nc.sync.dma_start(sin4_sb[:], sinq4[:])
                nc.sync.dma_start(cosk_sb[:], cosk[:])
                nc.sync.dma_start(sink_sb[:], sink[:])
                nc.sync.dma_start(kt_sb[:, :PAST], pastkt[:])
                nc.sync.dma_start(v_sb[:, : N_PV * 128], pastv[:])

                for st in range(N_ST):
                    q_ps = qkvps.tile([128, HPC * 128], F32, tag="qps")
                    kv_ps = qkvps.tile([128, 256], F32, tag="kvps")
                    for k in range(N_HK):
                        lhs = hs_sb[:, k * Q + st * 128: k * Q + (st + 1) * 128]
                        nc.tensor.matmul(
                            q_ps[:], lhs,
                            wq_sb[:, k * HPC * 128:(k + 1) * HPC * 128],
                            start=(k == 0), stop=(k == N_HK - 1),
                        )
                        nc.tensor.matmul(
                            kv_ps[:], lhs, wkv_sb[:, k * 256:(k + 1) * 256],
                            start=(k == 0), stop=(k == N_HK - 1),
                        )

                    # ---- RoPE q (4 heads batched) ----
                    csl = slice(st * HPC * HD, (st + 1) * HPC * HD)
                    q_sb = rpool.tile([128, HPC * HD], BF16, tag="qsb")
                    nc.scalar.copy(q_sb[:], q_ps[:])
                    qc = rpool.tile([128, HPC * HD], BF16, tag="qc")
                    nc.vector.tensor_mul(qc[:], q_sb[:], cos4_sb[:, csl])
                    qr = rpool.tile([128, HPC * HD], BF16, tag="qr")
                    q3 = q_sb[:].rearrange("p (h d) -> p h d", h=HPC)
                    qr3 = qr[:].rearrange("p (h d) -> p h d", h=HPC)
                    sn3 = sin4_sb[:, csl].rearrange("p (h d) -> p h d", h=HPC)
                    # rot-half: sin cols 0:64 pre-negated on host
                    nc.vector.tensor_mul(qr3[:, :, 0:64], q3[:, :, 64:128], sn3[:, :, 0:64])
                    nc.vector.tensor_mul(qr3[:, :, 64:128], q3[:, :, 0:64], sn3[:, :, 64:128])
                    qo = rpool.tile([128, HPC * HD], BF16, tag="qo")
                    nc.vector.tensor_add(qo[:], qc[:], qr[:])
                    tp = tps.tile([128, HPC * 128], BF16, tag="tp")
                    for h in range(HPC):
                        nc.tensor.transpose(
                            tp[:, h * 128:(h + 1) * 128],
                            qo[:, h * HD:(h + 1) * HD], ident[:],
                        )
                    nc.scalar.copy(
                        qt3[:, :, st * 128:(st + 1) * 128],
                        tp[:].rearrange("p (h q) -> p h q", h=HPC),
                    )

                    # ---- RoPE k + stash v ----
                    ksl = slice(st * HD, (st + 1) * HD)
                    k_sb = rpool.tile([128, HD], BF16, tag="ksb")
                    nc.scalar.copy(k_sb[:], kv_ps[:, 0:128])
                    nc.scalar.copy(
                        v_sb[:, (N_PV + st) * 128:(N_PV + st + 1) * 128],
                        kv_ps[:, 128:256],
                    )
                    kc = rpool.tile([128, HD], BF16, tag="kc")
                    nc.vector.tensor_mul(kc[:], k_sb[:], cosk_sb[:, ksl])
                    kr = rpool.tile([128, HD], BF16, tag="kr")
                    nc.vector.tensor_mul(kr[:, 0:64], k_sb[:, 64:128],
                                         sink_sb[:, st * HD: st * HD + 64])
                    nc.vector.tensor_mul(kr[:, 64:128], k_sb[:, 0:64],
                                         sink_sb[:, st * HD + 64:(st + 1) * HD])
                    ko = rpool.tile([128, HD], BF16, tag="ko")
                    nc.vector.tensor_add(ko[:], kc[:], kr[:])
                    tpk = tpkps.tile([128, HD], BF16, tag="tpkt")
                    nc.tensor.transpose(tpk[:], ko[:], ident[:])
                    nc.scalar.copy(
                        kt_sb[:, PAST + st * 128: PAST + (st + 1) * 128], tpk[:]
                    )

            # ---------------- wo prefetch (DMA overlaps stage 2) ----------
            wopool = tc.alloc_tile_pool(name="wo", bufs=1)
            wo_sb = wopool.tile([128, HPC * HID], BF16)
            for h in range(HPC):
                nc.sync.dma_start(
                    wo_sb[:, h * HID:(h + 1) * HID],
                    wot[h * 128:(h + 1) * 128, :],
                )

            # ---------------- stages 2+3: attention + o_proj --------------
            # Denominator folds and normalization tails are software-pipelined
            # TWO (g,h) iterations deep: folds(prev) are emitted after pair 1
            # of the next iteration, tails(prev) after pair 2, so the PE never
            # waits on the DVE denominator chain or the reciprocal.
            with (
                tc.tile_pool(name="dn", bufs=2) as dnpool,
                tc.tile_pool(name="rcdram", bufs=2, space="DRAM") as drpool,
                tc.tile_pool(name="aps", bufs=3, space="PSUM") as aps,
                tc.tile_pool(name="dps", bufs=1, space="PSUM") as dps,
            ):
                def fold(dn_bf, a_ps, g, h):
                    # partition-reduce the DVE elementwise partial on PE
                    ds_ps = dps.tile([1, GRP], F32, tag="dsum")
                    nc.tensor.matmul(
                        ds_ps[:], ones_col[:], dn_bf[:, 0:GRP],
                        start=True, stop=False,
                    )
                    nc.tensor.matmul(
                        ds_ps[:], ones_col[:], dn_bf[:, GRP:2 * GRP],
                        start=False, stop=True,
                    )
                    return (a_ps, ds_ps, g, h)

                def norm_tail(a_ps, ds_ps, g, h):
                    # 1/denominator, broadcast to 128 partitions, normalize.
                    # Broadcast runs on the (otherwise idle) GPSIMD engine so
                    # this whole chain stays off the PE.
                    rc_sb = dnpool.tile([1, GRP], F32, tag="recip")
                    nc.vector.reciprocal(rc_sb[:], ds_ps[:])
                    rc_dr = drpool.tile([1, GRP], F32, tag="rcd")
                    nc.sync.dma_start(rc_dr[:], rc_sb[:])
                    bc_sb = dnpool.tile([128, GRP], F32, tag="bcsb")
                    nc.sync.dma_start(bc_sb[:], rc_dr[:].partition_broadcast(128))
                    nc.vector.tensor_mul(
                        at_all[:, h * Q + g * GRP: h * Q + (g + 1) * GRP],
                        a_ps[:], bc_sb[:],
                    )

                with (
                    tc.tile_pool(name="mask", bufs=1) as mpool,
                    tc.tile_pool(name="pt", bufs=3) as ptpool,
                    tc.tile_pool(name="scps", bufs=2, space="PSUM") as scps,
                ):
                    m_sb = mpool.tile([128, 8 * GRP], BF16)
                    nc.sync.dma_start(m_sb[:], mask01[:])

                    fold_pending = None   # (dn_bf, a_ps, g, h)
                    tail_pending = None   # (a_ps, ds_ps, g, h)
                    for g in range(N_G):
                        nkt = _n_kt(g)
                        for h in range(HPC):
                            qsl = qt_all[:, h * Q + g * GRP: h * Q + (g + 1) * GRP]
                            a_ps = aps.tile([128, GRP], F32, tag="aacc")
                            dn_bf = None
                            for pi, kp in enumerate(range(0, nkt, 2)):
                                # two kv tiles per exp: scores in a 2-bank PSUM
                                s_ps = scps.tile([128, 2 * GRP], F32, tag="sps")
                                nc.tensor.matmul(
                                    s_ps[:, 0:GRP],
                                    kt_sb[:, kp * 128:(kp + 1) * 128],
                                    qsl, start=True, stop=True,
                                )
                                nc.tensor.matmul(
                                    s_ps[:, GRP:2 * GRP],
                                    kt_sb[:, (kp + 1) * 128:(kp + 2) * 128],
                                    qsl, start=True, stop=True,
                                )
                                pt = ptpool.tile([128, 2 * GRP], BF16, tag="pt")
                                nc.scalar.activation(
                                    pt[:], s_ps[:],
                                    mybir.ActivationFunctionType.Exp,
                                    bias=shift_sb[:], scale=1.0,
                                )
                                if _boundary(g, kp):
                                    b = 4 * g + (kp - (24 + 4 * g))
                                    nc.vector.tensor_mul(
                                        pt[:], pt[:],
                                        m_sb[:, b * GRP:(b + 2) * GRP],
                                    )
                                nc.tensor.matmul(
                                    a_ps[:], v_sb[:, kp * 128:(kp + 1) * 128],
                                    pt[:, 0:GRP],
                                    start=(kp == 0), stop=False,
                                )
                                nc.tensor.matmul(
                                    a_ps[:], v_sb[:, (kp + 1) * 128:(kp + 2) * 128],
                                    pt[:, GRP:2 * GRP],
                                    start=False, stop=(kp == nkt - 2),
                                )
                                # denominator partial on DVE (bf16)
                                if dn_bf is None:
                                    dn_bf = dnpool.tile(
                                        [128, 2 * GRP], BF16, tag="dnbf")
                                    nc.vector.tensor_copy(dn_bf[:], pt[:])
                                else:
                                    nc.vector.tensor_add(
                                        dn_bf[:], dn_bf[:], pt[:])
                                if pi == 1 and fold_pending is not None:
                                    tail_pending = fold(*fold_pending)
                                    fold_pending = None
                                if pi == 2 and tail_pending is not None:
                                    norm_tail(*tail_pending)
                                    tail_pending = None
                            fold_pending = (dn_bf, a_ps, g, h)

                # ---------------- stage 3: o_proj partial ----------------
                with (
                    tc.tile_pool(name="ostage", bufs=6) as ostpool,
                    tc.tile_pool(name="ops", bufs=4, space="PSUM") as opps,
                ):
                    for st in range(N_ST):
                        for n in range(HID // 512):
                            o_ps = opps.tile([128, 512], F32, tag="ops")
                            for h in range(HPC):
                                nc.tensor.matmul(
                                    o_ps[:],
                                    at_all[:, h * Q + st * 128:
                                           h * Q + (st + 1) * 128],
                                    wo_sb[:, h * HID + n * 512:
                                          h * HID + (n + 1) * 512],
                                    start=(h == 0), stop=(h == HPC - 1),
                                )
                            o_sb = ostpool.tile([128, 512], BF16, tag="osb")
                            if (st * (HID // 512) + n) % 2 == 0:
                                nc.scalar.copy(o_sb[:], o_ps[:])
                            else:
                                nc.vector.tensor_copy(o_sb[:], o_ps[:])
                            nc.sync.dma_start(
                                outp[st * 128:(st + 1) * 128,
                                     n * 512:(n + 1) * 512],
                                o_sb[:],
                            )
                            # last (g1,h3) fold/tail rides inside the first
                            # o_proj groups (they only need g0 slices)
                            if st == 0 and n == 0 and fold_pending is not None:
                                tail_pending = fold(*fold_pending)
                                fold_pending = None
                            if st == 0 and n == 2 and tail_pending is not None:
                                norm_tail(*tail_pending)
                                tail_pending = None
            wopool.release()
    return nc


def _pack_ktiles(a, tile_rows=128):
    """[R, C] -> [128, (R//128)*C] with k-tile kt at cols [kt*C:(kt+1)*C]."""
    r, c = a.shape
    n = r // tile_rows
    return np.ascontiguousarray(
        a.reshape(n, tile_rows, c).transpose(1, 0, 2).reshape(tile_rows, n * c)
    )


def _rope_tables(position_ids):
    pos = np.asarray(position_ids).reshape(-1).astype(np.float64)
    inv_freq = 1.0 / (ROPE_THETA ** (np.arange(0, HD, 2, dtype=np.float64) / HD))
    freqs = np.outer(pos, inv_freq)                      # [Q, 64]
    emb = np.concatenate([freqs, freqs], axis=-1)        # [Q, HD]
    return np.cos(emb).astype(np.float32), np.sin(emb).astype(np.float32)


def kernel(hidden_states, attention_mask, position_ids, past_k, past_v,
           Wq, Wk, Wv, Wo):
    global LAST_RESULTS
    bf = ml_dtypes.bfloat16

    hs = np.asarray(hidden_states, np.float32).reshape(Q, HID)
    mask = np.asarray(attention_mask, np.float32).reshape(Q, KV)
    cos, sin = _rope_tables(position_ids)

    scale = 1.0 / math.sqrt(HD)
    # sin tables: cols 0:64 negated (rotate-half sign), q tables pre-scaled
    sin_eff = sin.copy()
    sin_eff[:, :64] = -sin_eff[:, :64]

    def _per_st(tab):
        # [Q, HD] -> [128, N_ST*HD]: seq tile st at cols [st*HD:(st+1)*HD]
        return _pack_ktiles(tab)

    cosq4_t = _per_st(np.concatenate([cos * scale] * HPC, axis=1))
    sinq4_t = _per_st(np.concatenate([sin_eff * scale] * HPC, axis=1))
    cosk_t = _per_st(cos).astype(bf)
    sink_t = _per_st(sin_eff).astype(bf)
    cosq4_t = cosq4_t.astype(bf)
    sinq4_t = sinq4_t.astype(bf)

    # boundary 0/1 mask tiles: b=0..3 -> (g=0, kt=24+b); b=4..7 -> (g=1, kt=28+b-4)
    keep = (mask.T == 0.0).astype(np.float32)            # [KV, Q]
    mblocks = []
    for g in range(N_G):
        for kt in range(24 + 4 * g, 28 + 4 * g):
            mblocks.append(keep[kt * 128:(kt + 1) * 128, g * GRP:(g + 1) * GRP])
    mask01_t = np.concatenate(mblocks, axis=1).astype(bf)  # [128, 8*GRP]

    hst = np.ascontiguousarray(hs.T).astype(bf)                    # [4096, 1024]

    nc = _build_program()
    in_maps = []
    for c in range(NCORES):
        qs = slice(c * HPC * HD, (c + 1) * HPC * HD)
        ks = slice(c * HD, (c + 1) * HD)
        wq_c = np.ascontiguousarray(Wq[qs, :].T).astype(bf)        # [4096, 512]
        wk_c = np.ascontiguousarray(Wk[ks, :].T)                   # [4096, 128]
        wv_c = np.ascontiguousarray(Wv[ks, :].T)
        wkv_c = np.ascontiguousarray(
            np.concatenate([wk_c, wv_c], axis=1)).astype(bf)       # [4096, 256]
        pkt = np.ascontiguousarray(past_k[0, c].T).astype(bf)      # [128, 3072]
        pv = _pack_ktiles(np.ascontiguousarray(past_v[0, c])).astype(bf)
        wo_c = np.ascontiguousarray(Wo[:, qs].T).astype(bf)        # [512, 4096]
        in_maps.append({
            "hst": hst, "wqt": wq_c, "wkvt": wkv_c, "pastkt": pkt,
            "pastv": pv, "mask01": mask01_t, "cosq4": cosq4_t,
            "sinq4": sinq4_t, "cosk": cosk_t, "sink": sink_t, "wot": wo_c,
        })

    res = run_bass_kernel_spmd(nc, in_maps, list(range(NCORES)))
    LAST_RESULTS = res
    out = np.zeros((Q, HID), np.float32)
    for c in range(NCORES):
        out += np.asarray(res.results[c]["outp"], dtype=np.float32)
    return out.reshape(B, Q, HID)
